# revision 4
# baseline (speedup 1.0000x reference)
"""Trainium2 Bass kernel v2: dense transformer block (B=2, T=2048, C=1024, H=16, HD=64).

Sharding: 2 batch groups x 4-way tensor parallel (4 heads/core for attention,
token-split FFN). Differences vs v1:
  - LN1 computed feature-major from a host-transposed xT (bf16): stats via
    ones-matmuls (sum / sum-of-squares rows), mu/rstd broadcast back across
    partitions with K=1 PE matmuls, normalize as two bf16 DVE passes + one
    fused g/beta tensor_scalar. No PE transposes in phase A.
  - QKV head-pairs merged into single 128-wide lhsT matmuls (FWL-eligible),
    ic-outer loop so each weight load serves 4 query chunks.
  - ReduceScatter payload in bf16 (half the collective time), issued
    ascending so early chunks' collectives hide under later attention.
  - FFN split: pass A covers token tiles st0-2 (N=384) and full rows for
    ht>=16 once RS3 has landed; pass B catches up st3 / tt3. FFN starts
    right after the last proj instead of waiting for the last collective.
"""

import os
import sys

if "/opt/trn_rl_repo" not in sys.path:
    sys.path.insert(0, "/opt/trn_rl_repo")

import contextlib
import math

import ml_dtypes
import numpy as np

import concourse.bass as bass
import concourse.mybir as mybir
import concourse.tile as tile
from concourse import bacc
from concourse.bass_utils import run_bass_kernel_spmd
from concourse.masks import make_identity

try:
    from antenv import axon_hooks as _ah  # noqa: F401
except ImportError:
    import types as _types

    _shim = _types.ModuleType("antenv.axon_hooks")
    _shim._hook = None
    _shim.set_axon_ntff_profile_hook = lambda h: setattr(_shim, "_hook", h)
    _shim.get_axon_ntff_profile_hook = lambda: _shim._hook
    sys.modules["antenv.axon_hooks"] = _shim
    try:
        if "/root/.axon_site" not in sys.path:
            sys.path.insert(0, "/root/.axon_site")
        from trn_agent_boot.trn_boot import _ntff_profile_via_ctypes

        _shim.set_axon_ntff_profile_hook(
            _ntff_profile_via_ctypes("/opt/axon/libaxon_pjrt.so")
        )
    except Exception:
        pass

AF = mybir.ActivationFunctionType
ALU = mybir.AluOpType
FP32 = mybir.dt.float32
BF16 = mybir.dt.bfloat16
INT32 = mybir.dt.int32

P = 128
QCH = 512  # query chunk
KG = 2  # k-tiles batched per exp() call


def build_block(T=2048, C=1024, NHL=4, F=4096, GC=4, eps=1e-5, n_cores=8):
    HD = 64
    DL = NHL * HD  # 256
    NPAIR = NHL // 2
    NT = T // P  # 16
    NCc = C // P  # 8
    NQC = T // QCH  # 4
    KPC = QCH // P  # 4
    TSH = T // GC  # 512
    NST = TSH // P  # 4
    NHT = F // P  # 32
    HT_SPLIT = 16  # ht >= this use full-width FFN1 in pass A
    scale = 1.0 / math.sqrt(HD)

    groups = [list(range(g * GC, (g + 1) * GC)) for g in range(n_cores // GC)]

    nc = bacc.Bacc(
        "TRN2", target_bir_lowering=False, num_devices=n_cores, debug=False
    )

    # ---- I/O ----
    xt_d = nc.dram_tensor("xt", [C, T], BF16, kind="ExternalInput")
    x_shard = nc.dram_tensor("x_shard", [TSH, C], FP32, kind="ExternalInput")
    wq_d = nc.dram_tensor("wq", [C, DL], BF16, kind="ExternalInput")
    wk_d = nc.dram_tensor("wk", [C, DL], BF16, kind="ExternalInput")
    wv_d = nc.dram_tensor("wv", [C, DL], BF16, kind="ExternalInput")
    wp_d = nc.dram_tensor("wp", [DL, C], BF16, kind="ExternalInput")
    w1_d = nc.dram_tensor("w1", [C, F], BF16, kind="ExternalInput")
    w2_d = nc.dram_tensor("w2", [F, C], BF16, kind="ExternalInput")
    b1_d = nc.dram_tensor("b1r", [P, NHT], FP32, kind="ExternalInput")
    bp_d = nc.dram_tensor("bp", [C], BF16, kind="ExternalInput")
    b2_d = nc.dram_tensor("b2", [C], BF16, kind="ExternalInput")
    g1_d = nc.dram_tensor("g1r", [P, NCc], FP32, kind="ExternalInput")
    be1_d = nc.dram_tensor("be1r", [P, NCc], FP32, kind="ExternalInput")
    g2_d = nc.dram_tensor("g2r", [P, NCc], FP32, kind="ExternalInput")
    be2_d = nc.dram_tensor("be2r", [P, NCc], FP32, kind="ExternalInput")
    mask_d = nc.dram_tensor("maskr", [P, P], BF16, kind="ExternalInput")
    out_d = nc.dram_tensor("out", [TSH, C], FP32, kind="ExternalOutput")

    with tile.TileContext(nc) as tc, contextlib.ExitStack() as est:
        big = est.enter_context(tc.tile_pool(name="big", bufs=18))
        sqp = est.enter_context(tc.tile_pool(name="sqp", bufs=2))
        vp = est.enter_context(tc.tile_pool(name="vp", bufs=NT))
        h2Tp = est.enter_context(tc.tile_pool(name="h2Tp", bufs=NCc))
        x2p = est.enter_context(tc.tile_pool(name="x2p", bufs=NST))
        attp = est.enter_context(tc.tile_pool(name="attp", bufs=3))
        tok = est.enter_context(tc.tile_pool(name="tok", bufs=3))
        exps = est.enter_context(tc.tile_pool(name="exps", bufs=3))
        w1p = est.enter_context(tc.tile_pool(name="w1p", bufs=2))
        w2p = est.enter_context(tc.tile_pool(name="w2p", bufs=5))
        obp = est.enter_context(tc.tile_pool(name="obp", bufs=1))
        sing = est.enter_context(tc.tile_pool(name="sing", bufs=1))
        psW = est.enter_context(tc.tile_pool(name="psW", bufs=2, space="PSUM"))
        psN = est.enter_context(tc.tile_pool(name="psN", bufs=2, space="PSUM"))
        psF = est.enter_context(tc.tile_pool(name="psF", bufs=1, space="PSUM"))
        dram = est.enter_context(tc.tile_pool(name="dram", bufs=1, space="DRAM"))

        # ---- singles ----
        ident = sing.tile([P, P], FP32, tag="ident", name="ident")
        make_identity(nc, ident)
        eps_t = sing.tile([P, 1], FP32, tag="eps", name="eps")
        nc.vector.memset(eps_t, eps)
        ones_t = sing.tile([1, HD], BF16, tag="ones", name="ones")
        nc.vector.memset(ones_t, 1.0)
        ones_col = sing.tile([P, 1], BF16, tag="onesc", name="onesc")
        nc.vector.memset(ones_col, 1.0)
        ones_rows = sing.tile([33, P], BF16, tag="onesr", name="onesr")
        nc.vector.memset(ones_rows, 1.0)

        rs_in_t = [
            dram.tile([QCH, C], BF16, tag=f"rsin{k}", name=f"rsin{k}")
            for k in range(NQC)
        ]
        rs_out_t = [
            dram.tile([QCH // GC, C], BF16, tag=f"rsout{k}", name=f"rsout{k}")
            for k in range(NQC)
        ]


        def mm2(out, lhsT, rhs, start, stop):
            """K=128 matmul split into two K=64 row-group halves: LDWEIGHTS
            for one half pulls ahead while the other half's matmul streams,
            and the halves run concurrently (distinct row groups)."""
            nc.tensor.matmul(
                out, lhsT=lhsT[0:64, :], rhs=rhs[0:64, :],
                start=start, stop=False, tile_position=(0, 0),
            )
            nc.tensor.matmul(
                out, lhsT=lhsT[64:128, :], rhs=rhs[64:128, :],
                start=False, stop=stop, tile_position=(64, 0),
                skip_group_check=True,
            )

        # ================= Phase A: feature-major LN1 =================
        xt = []
        for ic in range(NCc):
            t = big.tile([P, T], BF16, tag="big", name=f"xt{ic}")
            nc.sync.dma_start(t, xt_d[ic * P : (ic + 1) * P, :])
            xt.append(t)

        # weight/bias DMAs issued after xt so the input tiles arrive first
        wq_sb = sing.tile([P, NCc, DL], BF16, tag="wq", name="wq")
        nc.sync.dma_start(wq_sb, wq_d.rearrange("(n p) m -> p n m", p=P))
        wk_sb = sing.tile([P, NCc, DL], BF16, tag="wk", name="wk")
        nc.sync.dma_start(wk_sb, wk_d.rearrange("(n p) m -> p n m", p=P))
        wv_sb = sing.tile([P, NCc, DL], BF16, tag="wv", name="wv")
        nc.sync.dma_start(wv_sb, wv_d.rearrange("(n p) m -> p n m", p=P))
        wp_sb = sing.tile([P, DL // P, C], BF16, tag="wp", name="wp")
        nc.sync.dma_start(wp_sb, wp_d.rearrange("(n p) m -> p n m", p=P))
        mask_sb = sing.tile([P, P], BF16, tag="mask", name="mask")
        nc.sync.dma_start(mask_sb, mask_d[:])
        b1_sb = sing.tile([P, NHT], FP32, tag="b1", name="b1")
        nc.sync.dma_start(b1_sb, b1_d[:])
        g1_sb = sing.tile([P, NCc], FP32, tag="g1", name="g1")
        nc.sync.dma_start(g1_sb, g1_d[:])
        be1_sb = sing.tile([P, NCc], FP32, tag="be1", name="be1")
        nc.sync.dma_start(be1_sb, be1_d[:])
        g2_sb = sing.tile([P, NCc], FP32, tag="g2", name="g2")
        nc.sync.dma_start(g2_sb, g2_d[:])
        be2_sb = sing.tile([P, NCc], FP32, tag="be2", name="be2")
        nc.sync.dma_start(be2_sb, be2_d[:])
        bp_bc = sing.tile([P, C], BF16, tag="bpbc", name="bpbc")
        bp_ap = bp_d[:]
        nc.sync.dma_start(
            bp_bc,
            bass.AP(tensor=bp_ap.tensor, offset=bp_ap.offset, ap=[[0, P]] + list(bp_ap.ap)),
        )
        b2_bc = sing.tile([P, C], BF16, tag="b2bc", name="b2bc")
        b2_ap = b2_d[:]
        nc.sync.dma_start(
            b2_bc,
            bass.AP(tensor=b2_ap.tensor, offset=b2_ap.offset, ap=[[0, P]] + list(b2_ap.ap)),
        )


        NSC = T // 1024  # stat psum tiles (2)
        psS = [psW.tile([P, 1024], FP32, tag="psW", name=f"psS{j}") for j in range(NSC)]
        for ic in range(NCc):
            sq = sqp.tile([P, T], BF16, tag="sq", name=f"sq{ic}")
            nc.scalar.activation(out=sq, in_=xt[ic], func=AF.Square, scale=1.0)
            for j in range(NSC):
                for h in range(2):
                    col = j * 1024 + h * 512
                    nc.tensor.matmul(
                        psS[j][0:1, h * 512 : h * 512 + 512],
                        lhsT=ones_col,
                        rhs=xt[ic][:, col : col + 512],
                        start=(ic == 0),
                        stop=(ic == NCc - 1),
                        tile_position=(0, 0),
                    )
                    nc.tensor.matmul(
                        psS[j][32:33, h * 512 : h * 512 + 512],
                        lhsT=ones_col,
                        rhs=sq[:, col : col + 512],
                        start=(ic == 0),
                        stop=(ic == NCc - 1),
                        tile_position=(0, 32),
                        skip_group_check=True,
                    )

        # mu/msq rows -> SBUF (partition 0 holds sums, partition 32 sumsqs;
        # each ACT copy has a single SBUF operand so start partitions are free)
        rows_f = sing.tile([33, T], BF16, tag="rowsf", name="rows_f")
        for j in range(NSC):
            nc.scalar.activation(
                out=rows_f[0:1, j * 1024 : (j + 1) * 1024],
                in_=psS[j][0:1, :],
                func=AF.Identity,
                scale=1.0 / C,
            )
            nc.scalar.activation(
                out=rows_f[32:33, j * 1024 : (j + 1) * 1024],
                in_=psS[j][32:33, :],
                func=AF.Identity,
                scale=1.0 / C,
            )

        # per 512-token chunk: broadcast mu and msq across partitions, then
        # compute rstd / mu*rstd full-width (PSUM operands are exempt from
        # the same-start-partition rule, SBUF outputs all live at base 0)
        rstd_bc = big.tile([P, T], BF16, tag="big", name="rstd_bc")
        murstd_bc = big.tile([P, T], BF16, tag="big", name="murstd_bc")
        for c4 in range(T // 512):
            j, h = c4 // 2, c4 % 2
            col = c4 * 512
            bc_t = psW.tile([P, 1024], FP32, tag="psW", name=f"bct{c4}")
            nc.tensor.matmul(
                bc_t[:, 0:512],
                lhsT=ones_rows[0:1, :],
                rhs=rows_f[0:1, col : col + 512],
                start=True,
                stop=True,
            )
            nc.tensor.matmul(
                bc_t[:, 512:1024],
                lhsT=ones_rows[32:33, :],
                rhs=rows_f[32:33, col : col + 512],
                start=True,
                stop=True,
            )
            mu2 = sqp.tile([P, 512], FP32, tag="scr", name="mu2", bufs=2)
            nc.scalar.activation(out=mu2, in_=bc_t[:, 0:512], func=AF.Square)
            nc.vector.tensor_sub(bc_t[:, 512:1024], bc_t[:, 512:1024], mu2)
            nc.scalar.activation(
                out=bc_t[:, 512:1024],
                in_=bc_t[:, 512:1024],
                func=AF.Sqrt,
                bias=eps_t,
                scale=1.0,
            )
            rst = sqp.tile([P, 512], FP32, tag="scr", name="rst", bufs=2)
            nc.vector.reciprocal_approx_fast(out=rst, in_=bc_t[:, 512:1024])
            nc.vector.tensor_copy(rstd_bc[:, col : col + 512], rst)
            nc.vector.tensor_mul(murstd_bc[:, col : col + 512], bc_t[:, 0:512], rst)

        # normalize: hT = (xT*rstd - mu*rstd) * g1[c] + be1[c]
        hT = []
        for ic in range(NCc):
            h_t = big.tile([P, T], BF16, tag="big", name=f"hT{ic}")
            nc.vector.tensor_mul(h_t, xt[ic], rstd_bc)
            nc.vector.tensor_sub(h_t, h_t, murstd_bc)
            nc.vector.tensor_scalar(
                out=h_t,
                in0=h_t,
                scalar1=g1_sb[:, ic : ic + 1],
                scalar2=be1_sb[:, ic : ic + 1],
                op0=ALU.mult,
                op1=ALU.add,
            )
            hT.append(h_t)

        # ================= Phase B: QKV =================
        QT = [big.tile([P, T], BF16, tag="big", name=f"QT{pr}") for pr in range(NPAIR)]
        KT = [big.tile([P, T], BF16, tag="big", name=f"KT{pr}") for pr in range(NPAIR)]
        V4 = []

        HD1 = HD + 1  # per-head V columns + ones column for the rowsum row

        def v_block(it0, n):
            for it in range(it0, it0 + n):
                ps = psN.tile([P, 512], FP32, tag="psN", name="vps")
                for ic in range(NCc):
                    nc.tensor.matmul(
                        ps[:, 0:DL],
                        lhsT=hT[ic][:, it * P : (it + 1) * P],
                        rhs=wv_sb[:, ic, :],
                        start=(ic == 0),
                        stop=(ic == NCc - 1),
                    )
                v_t = vp.tile([P, NHL * HD1], BF16, tag="v", name="v")
                nc.vector.memset(v_t, 1.0)
                v4 = v_t.rearrange("p (h d) -> p h d", h=NHL)
                p4 = ps[:, 0:DL].rearrange("p (h d) -> p h d", h=NHL)
                nc.vector.tensor_copy(v4[:, :, 0:HD], p4)
                V4.append(v_t)

        def qk_block(dst, w_sb, pr):
            ps2 = [psW.tile([P, 1024], FP32, tag="psW", name="qkps") for _ in range(2)]
            for ic in range(NCc):
                for ch in range(NQC):
                    nc.tensor.matmul(
                        ps2[ch // 2][:, (ch % 2) * 512 : (ch % 2) * 512 + 512],
                        lhsT=w_sb[:, ic, pr * P : (pr + 1) * P],
                        rhs=hT[ic][:, ch * QCH : (ch + 1) * QCH],
                        start=(ic == 0),
                        stop=(ic == NCc - 1),
                    )
            for j in range(2):
                nc.vector.tensor_copy(
                    dst[:, j * 1024 : (j + 1) * 1024], ps2[j]
                )

        qk_block(QT[0], wq_sb, 0)
        v_block(0, 4)
        qk_block(KT[0], wk_sb, 0)
        v_block(4, 4)
        qk_block(QT[1], wq_sb, 1)
        v_block(8, 4)
        qk_block(KT[1], wk_sb, 1)
        v_block(12, 4)

        # ============ Phase C+D: attention + proj + RS per q-chunk ============
        h2T = [h2Tp.tile([P, TSH], BF16, tag="h2T", name="h2T") for _ in range(NCc)]
        x2pb = [None] * NST

        # LayerNorm helper for phase E (token-major)
        fmax = math.gcd(512, C)
        nsub = C // fmax

        def ln_stats(x_t, pool, tagp):
            stats = pool.tile([P, nsub, 6], FP32, tag=f"{tagp}_st", name=f"{tagp}_st")
            xr = x_t.rearrange("p (n f) -> p n f", n=nsub)
            for s in range(nsub):
                nc.vector.bn_stats(out=stats[:, s, :], in_=xr[:, s, :])
            mv = pool.tile([P, 2], FP32, tag=f"{tagp}_mv", name=f"{tagp}_mv")
            nc.vector.bn_aggr(out=mv, in_=stats)
            nc.scalar.activation(
                out=mv[:, 1:2], in_=mv[:, 1:2], func=AF.Sqrt, bias=eps_t, scale=1.0
            )
            nc.vector.reciprocal(out=mv[:, 1:2], in_=mv[:, 1:2])
            return mv

        def phase_e(st):
            r_t = tok.tile([P, C], BF16, tag="rb", name="rt", bufs=2)
            nc.sync.dma_start(r_t, rs_out_t[st][:])
            xs_t = tok.tile([P, C], FP32, tag="tb", name="xst")
            nc.sync.dma_start(xs_t, x_shard[st * P : (st + 1) * P, :])
            x2_t = tok.tile([P, C], FP32, tag="tb", name="x2t")
            nc.vector.tensor_add(x2_t, xs_t, r_t)
            nc.vector.tensor_add(x2_t, x2_t, bp_bc)
            mv = ln_stats(x2_t, tok, "ln2")
            h2_t = tok.tile([P, C], FP32, tag="tb", name="h2t")
            nc.vector.tensor_scalar(
                out=h2_t,
                in0=x2_t,
                scalar1=mv[:, 0:1],
                scalar2=mv[:, 1:2],
                op0=ALU.subtract,
                op1=ALU.mult,
            )
            xb = x2p.tile([P, C], BF16, tag="x2pb", name="x2pb")
            nc.vector.tensor_add(xb, x2_t, b2_bc)
            x2pb[st] = xb
            for ic in range(NCc):
                ps = psN.tile([P, P], FP32, tag="psN", name="psE")
                nc.tensor.transpose(ps, h2_t[:, ic * P : (ic + 1) * P], ident)
                dst = h2T[ic][:, st * P : (st + 1) * P]
                nc.vector.tensor_scalar(
                    out=dst,
                    in0=ps,
                    scalar1=g2_sb[:, ic : ic + 1],
                    scalar2=be2_sb[:, ic : ic + 1],
                    op0=ALU.mult,
                    op1=ALU.add,
                )

        for ch in range(NQC):
            n_kt = (ch + 1) * KPC
            attT = []
            for pr in range(NPAIR):
                att_ps = [
                    psN.tile([P, QCH], FP32, tag="psN", name="attps") for _ in range(2)
                ]
                # software-pipelined per-k-tile loop: scores for k-tile i
                # are issued 2 iterations ahead of attV(i), so the PE FIFO
                # never blocks waiting for exp(i) on the scalar engine.
                def sc_exp(i):
                    sp_pool = psF if i % 3 == 2 else psW
                    s2 = sp_pool.tile([P, 1024], FP32, tag=sp_pool.name, name="s2")
                    for pos in range(2):
                        nc.tensor.matmul(
                            s2[:, pos * QCH : (pos + 1) * QCH],
                            lhsT=KT[pr][64 * pos : 64 * pos + 64, i * P : (i + 1) * P],
                            rhs=QT[pr][
                                64 * pos : 64 * pos + 64, ch * QCH : (ch + 1) * QCH
                            ],
                            start=True,
                            stop=True,
                            tile_position=(64 * pos, 0),
                        )
                    e2 = exps.tile([P, 1024], BF16, tag="e", name="e", bufs=4)
                    nc.scalar.activation(out=e2, in_=s2, func=AF.Exp, scale=scale)
                    jd = i - ch * KPC
                    if 0 <= jd < KPC:
                        for pos in range(2):
                            if jd > 0:
                                nc.vector.memset(
                                    e2[:, pos * QCH : pos * QCH + jd * P], 0.0
                                )
                            tri = slice(pos * QCH + jd * P, pos * QCH + (jd + 1) * P)
                            nc.vector.tensor_mul(e2[:, tri], e2[:, tri], mask_sb)
                    return e2

                def att_v(i, e2):
                    for pos in range(2):
                        lh = 2 * pr + pos
                        nc.tensor.matmul(
                            att_ps[pos][0:65, :],
                            lhsT=V4[i][:, lh * HD1 : (lh + 1) * HD1],
                            rhs=e2[:, pos * QCH : (pos + 1) * QCH],
                            start=(i == 0),
                            stop=(i == n_kt - 1),
                        )

                e_hist = {}
                for i in range(n_kt):
                    e_hist[i] = sc_exp(i)
                    if i >= 2:
                        att_v(i - 2, e_hist.pop(i - 2))
                for i in (n_kt - 2, n_kt - 1):
                    att_v(i, e_hist.pop(i))
                # evacuate att+rowsum to SBUF early (frees psN for next pr)
                araw = exps.tile([P, QCH], BF16, tag="araw", name="araw", bufs=2)
                rs_raw = []
                for pos in range(2):
                    nc.vector.tensor_copy(
                        araw[64 * pos : 64 * pos + 64, :], att_ps[pos][0:64, :]
                    )
                    rr = exps.tile([1, QCH], BF16, tag="rraw", name="rraw", bufs=2)
                    nc.vector.tensor_copy(rr, att_ps[pos][64:65, :])
                    rs_raw.append(rr)
                # broadcast rowsums, reciprocal, apply
                bc_ps = psW.tile([P, KG * QCH], FP32, tag="psW", name="bcn")
                for pos in range(2):
                    nc.tensor.matmul(
                        bc_ps[64 * pos : 64 * pos + 64, 0:QCH],
                        lhsT=ones_t,
                        rhs=rs_raw[pos],
                        start=True,
                        stop=True,
                        tile_position=(0, 64 * pos),
                    )
                rec_bc = exps.tile([P, QCH], FP32, tag="rec", name="recbc", bufs=2)
                nc.vector.reciprocal_approx_fast(out=rec_bc, in_=bc_ps[:, 0:QCH])
                at = attp.tile([P, QCH], BF16, tag="attT", name="attT")
                for pos in range(2):
                    rows = slice(64 * pos, 64 * pos + 64)
                    nc.vector.tensor_mul(at[rows, :], araw[rows, :], rec_bc[rows, :])
                attT.append(at)

            # proj partials (token-major) -> rs_in (bf16)
            for tt in range(KPC):
                pj = psW.tile([P, 1024], FP32, tag="psW", name="pj")
                for oc in range(2):
                    for pr in range(NPAIR):
                        nc.tensor.matmul(
                            pj[:, oc * 512 : (oc + 1) * 512],
                            lhsT=attT[pr][:, tt * P : (tt + 1) * P],
                            rhs=wp_sb[:, pr, oc * 512 : (oc + 1) * 512],
                            start=(pr == 0),
                            stop=(pr == NPAIR - 1),
                        )
                pj_sb = exps.tile([P, C], BF16, tag="pj", name="pjsb", bufs=2)
                nc.vector.tensor_copy(pj_sb, pj)
                nc.sync.dma_start(rs_in_t[ch][tt * P : (tt + 1) * P, :], pj_sb)

            nc.gpsimd.collective_compute(
                "ReduceScatter",
                ALU.add,
                replica_groups=groups,
                ins=[rs_in_t[ch][:].opt()],
                outs=[rs_out_t[ch][:].opt()],
            )

        # ============ Phase E for st0-2 (RS0-2 have landed) ============
        phase_e(0)
        phase_e(1)
        phase_e(2)

        # ================= Phase F: FFN =================
        # hidT: 8 big bufs of [P, T]; 4 h-tiles of [P, TSH] per buf
        HPB = T // TSH  # h-tiles per big buf (4)
        hidT = [big.tile([P, T], BF16, tag="big", name=f"hid{j}") for j in range(NHT // HPB)]

        def hid_slice(ht, t0, tlen):
            j, o = ht // HPB, ht % HPB
            return hidT[j][:, o * TSH + t0 : o * TSH + t0 + tlen]

        grp = [None] * NST
        grp[0] = psW.tile([P, 1024], FP32, tag="psW", name="grp0")
        grp[1] = psW.tile([P, 1024], FP32, tag="psW", name="grp1")
        grp[2] = psF.tile([P, 1024], FP32, tag="psF", name="grp2")

        w2_tiles = {}

        # ---- pass A: st0-2 for all ht (full width for ht >= HT_SPLIT) ----
        for h4 in range(NHT // 4):
            w1t = w1p.tile([P, NCc, 512], BF16, tag="w1t", name="w1t")
            for ic in range(NCc):
                nc.sync.dma_start(
                    w1t[:, ic, :],
                    w1_d[ic * P : (ic + 1) * P, h4 * 512 : (h4 + 1) * 512],
                )
            for hh in range(4):
                ht = h4 * 4 + hh
                if ht == HT_SPLIT:
                    phase_e(3)
                wide = ht >= HT_SPLIT
                ncol = 512 if wide else 384
                hid_ps = psN.tile([P, 512], FP32, tag="psN", name="hidps")
                for ic in range(NCc):
                    nc.tensor.matmul(
                        hid_ps[:, 0:ncol],
                        lhsT=w1t[:, ic, hh * P : (hh + 1) * P],
                        rhs=h2T[ic][:, 0:ncol],
                        start=(ic == 0),
                        stop=(ic == NCc - 1),
                    )
                nc.scalar.activation(
                    out=hid_slice(ht, 0, ncol),
                    in_=hid_ps[:, 0:ncol],
                    func=AF.Relu,
                    bias=b1_sb[:, ht : ht + 1],
                    scale=1.0,
                )
                w2t = w2p.tile([P, C], BF16, tag="w2t", name="w2t")
                nc.sync.dma_start(w2t, w2_d[ht * P : (ht + 1) * P, :])
                w2_tiles[ht] = w2t
                for tt in range(3):
                    for oc in range(2):
                        nc.tensor.matmul(
                            grp[tt][:, oc * 512 : (oc + 1) * 512],
                            lhsT=hid_slice(ht, tt * P, P),
                            rhs=w2t[:, oc * 512 : (oc + 1) * 512],
                            start=(ht == 0),
                            stop=(ht == NHT - 1),
                        )
        for tt in range(3):
            ob = obp.tile([P, C], FP32, tag="ob", name="ob")
            nc.vector.tensor_add(ob, grp[tt], x2pb[tt])
            nc.sync.dma_start(out_d[tt * P : (tt + 1) * P, :], ob)

        # ---- pass B: st3 catch-up (FFN1 for ht<HT_SPLIT) + FFN2 tt3 ----
        tc.tile_set_cur_wait(2.3)
        grp3 = psF.tile([P, 1024], FP32, tag="psF", name="grp3")
        for h4 in range(HT_SPLIT // 4):
            w1t = w1p.tile([P, NCc, 512], BF16, tag="w1t", name="w1tB")
            for ic in range(NCc):
                nc.sync.dma_start(
                    w1t[:, ic, :],
                    w1_d[ic * P : (ic + 1) * P, h4 * 512 : (h4 + 1) * 512],
                )
            for hh in range(4):
                ht = h4 * 4 + hh
                hid_ps = psN.tile([P, 512], FP32, tag="psN", name="hidpsB")
                for ic in range(NCc):
                    nc.tensor.matmul(
                        hid_ps[:, 0:P],
                        lhsT=w1t[:, ic, hh * P : (hh + 1) * P],
                        rhs=h2T[ic][:, 384:512],
                        start=(ic == 0),
                        stop=(ic == NCc - 1),
                    )
                nc.scalar.activation(
                    out=hid_slice(ht, 384, P),
                    in_=hid_ps[:, 0:P],
                    func=AF.Relu,
                    bias=b1_sb[:, ht : ht + 1],
                    scale=1.0,
                )
        tc.tile_set_cur_wait(2.4)
        # reversed processed order so the last-5 pass-A w2 tiles are reused
        # from the ring before any pass-B allocation evicts them
        processed = [b * 4 + hh for b in blocks for hh in range(4)]
        for k, ht in enumerate(reversed(processed)):
            if k < 5:
                w2t = w2_tiles[ht]  # still live in the ring
            else:
                w2t = w2p.tile([P, C], BF16, tag="w2t", name="w2tB")
                nc.sync.dma_start(w2t, w2_d[ht * P : (ht + 1) * P, :])
            for oc in range(2):
                nc.tensor.matmul(
                    grp3[:, oc * 512 : (oc + 1) * 512],
                    lhsT=hid_slice(ht, 3 * P, P),
                    rhs=w2t[:, oc * 512 : (oc + 1) * 512],
                    start=(k == 0),
                    stop=(k == NHT - 1),
                )
        tc.tile_set_cur_wait(2.5)
        ob = obp.tile([P, C], FP32, tag="ob", name="ob3")
        nc.vector.tensor_add(ob, grp3, x2pb[3])
        nc.sync.dma_start(out_d[3 * P : 4 * P, :], ob)

    nc.finalize()
    return nc


# ------------------------- host side -------------------------

_CACHE = {}
LAST_RESULTS = None


def make_in_maps(inputs, T=2048, C=1024, H=16, F=4096, GC=4, n_cores=8):
    HD = 64
    NHL = H // GC
    DL = NHL * HD
    TSH = T // GC
    NHT = F // P
    NCc = C // P
    bf = ml_dtypes.bfloat16

    x = np.asarray(inputs["x"], np.float32)
    Wq = np.asarray(inputs["Wq"], np.float32)
    Wk = np.asarray(inputs["Wk"], np.float32)
    Wv = np.asarray(inputs["Wv"], np.float32)
    Wp = np.asarray(inputs["Wp"], np.float32)
    bp = np.asarray(inputs["bp"], np.float32)
    W1 = np.asarray(inputs["W1"], np.float32)
    b1 = np.asarray(inputs["b1"], np.float32)
    W2 = np.asarray(inputs["W2"], np.float32)
    b2 = np.asarray(inputs["b2"], np.float32)
    g1 = np.asarray(inputs["g1"], np.float32)
    be1 = np.asarray(inputs["beta1"], np.float32)
    g2 = np.asarray(inputs["g2"], np.float32)
    be2 = np.asarray(inputs["beta2"], np.float32)

    maskr = np.triu(np.ones((P, P), np.float32)).astype(bf)
    b1r = np.ascontiguousarray(b1.reshape(NHT, P).T)
    g1r = np.ascontiguousarray(g1.reshape(NCc, P).T)
    be1r = np.ascontiguousarray(be1.reshape(NCc, P).T)
    g2r = np.ascontiguousarray(g2.reshape(NCc, P).T)
    be2r = np.ascontiguousarray(be2.reshape(NCc, P).T)
    w1b = W1.astype(bf)
    w2b = W2.astype(bf)

    NQC = T // QCH
    RPC = QCH // GC

    def shard_rows(g):
        return np.concatenate(
            [np.arange(k * QCH + g * RPC, k * QCH + (g + 1) * RPC) for k in range(NQC)]
        )

    in_maps = []
    for c in range(n_cores):
        b, g = c // GC, c % GC
        hsl = slice(g * NHL, (g + 1) * NHL)
        in_maps.append(
            {
                "xt": np.ascontiguousarray(x[b].T).astype(bf),
                "x_shard": np.ascontiguousarray(x[b][shard_rows(g)]),
                "wq": np.ascontiguousarray(
                    Wq[hsl].transpose(1, 0, 2).reshape(C, DL)
                ).astype(bf),
                "wk": np.ascontiguousarray(
                    Wk[hsl].transpose(1, 0, 2).reshape(C, DL)
                ).astype(bf),
                "wv": np.ascontiguousarray(
                    Wv[hsl].transpose(1, 0, 2).reshape(C, DL)
                ).astype(bf),
                "wp": np.ascontiguousarray(Wp[g * DL : (g + 1) * DL]).astype(bf),
                "w1": w1b,
                "w2": w2b,
                "b1r": b1r,
                "bp": bp.astype(bf),
                "b2": b2.astype(bf),
                "g1r": g1r,
                "be1r": be1r,
                "g2r": g2r,
                "be2r": be2r,
                "maskr": maskr,
            }
        )
    return in_maps


def kernel(**inputs) -> np.ndarray:
    global LAST_RESULTS
    B, T, C = inputs["x"].shape
    H = inputs["Wq"].shape[0]
    F = inputs["W1"].shape[1]
    GC = 4
    n_cores = 8
    key = (T, C, H, F)
    if key not in _CACHE:
        _CACHE[key] = build_block(T=T, C=C, NHL=H // GC, F=F, GC=GC, n_cores=n_cores)
    nc = _CACHE[key]
    in_maps = make_in_maps(inputs, T=T, C=C, H=H, F=F, GC=GC, n_cores=n_cores)
    res = run_bass_kernel_spmd(nc, in_maps, core_ids=list(range(n_cores)))
    LAST_RESULTS = res
    out = np.empty((B, T, C), np.float32)
    NQC = T // QCH
    RPC = QCH // GC
    for c in range(n_cores):
        b, g = c // GC, c % GC
        sh = res.results[c]["out"]
        for k in range(NQC):
            out[b, k * QCH + g * RPC : k * QCH + (g + 1) * RPC] = sh[
                k * RPC : (k + 1) * RPC
            ]
    return out
